# revision 2
# baseline (speedup 1.0000x reference)
"""CoAttention kernel for 8 Trainium2 NeuronCores.

Problem: S, D: [8, 2048, 1024] f32.
  G_b = D_b @ S_b^T                         [2048, 2048]
  co_D = D + rowsoftmax(G) @ S
  co_S = S + rowsoftmax(G^T) @ D
Data-parallel over batch: one batch per core, same NEFF on all 8 cores.

Per-core algorithm (all loops fully unrolled, Tile-scheduled):
  Phase A: load S; keep S^T (fp32r, for stage-1 rhs) and S_nat (bf16,
           stage-2 rhs) in SBUF.
  Phase B (per 128-row l-block): load D block, PE-transpose to fp32r
           D^T tiles; 8x accumulate fp32r matmuls -> G block; row max;
           exp (ACT, accum_out gives rowsum); also PE-transpose the
           fp32 G block out to a DRAM G^T buffer for phase C.
           Then O_D = (E/rowsum) @ S via PE-transposed bf16 E tiles,
           + D residual -> co_D.
  Phase C (per 128-row m-block of G^T): load G^T rows, col max, exp,
           O_S = (E2/colsum) @ D via PE-transposed bf16 E2 tiles,
           + S residual -> co_S.
fp32 logits throughout; bf16 only on post-softmax attention weights and
stage-2 operands. Softmax uses exact per-row / per-column maxes.
"""

import numpy as np

P = 128
T = 2048
DH = 1024
LT = T // P     # 16 token blocks per side
KD = DH // P    # 8 contraction blocks
NTILE = 512     # matmul moving free dim

_CACHE = {}


def _build_nc():
    import concourse.bass as bass
    import concourse.mybir as mybir
    import concourse.tile as tile
    from concourse import bacc
    from concourse.masks import make_identity

    dt = mybir.dt
    f32, f32r, bf16 = dt.float32, dt.float32r, dt.bfloat16
    AX = mybir.AxisListType.X
    EXP = mybir.ActivationFunctionType.Exp
    COPY = mybir.ActivationFunctionType.Copy

    nc = bacc.Bacc("TRN2", target_bir_lowering=False, debug=False)

    S_ap = nc.dram_tensor("S", [T, DH], f32, kind="ExternalInput").ap()
    D_ap = nc.dram_tensor("D", [T, DH], f32, kind="ExternalInput").ap()
    coD_ap = nc.dram_tensor("co_D", [T, DH], f32, kind="ExternalOutput").ap()
    coS_ap = nc.dram_tensor("co_S", [T, DH], f32, kind="ExternalOutput").ap()

    with tile.TileContext(nc) as tc:
        with (
            tc.tile_pool(name="consts", bufs=1) as consts,
            tc.tile_pool(name="big", bufs=1) as big,
            tc.tile_pool(name="dram", bufs=1, space="DRAM") as dram,
            tc.tile_pool(name="stage", bufs=3) as stage,
            tc.tile_pool(name="gtld", bufs=2) as gtld,
            tc.tile_pool(name="dtp", bufs=2) as dtp,
            tc.tile_pool(name="gsb", bufs=2) as gsb,
            tc.tile_pool(name="epool", bufs=2) as epool,
            tc.tile_pool(name="etp", bufs=3) as etp,
            tc.tile_pool(name="gtsb", bufs=3) as gtsb,
            tc.tile_pool(name="outp", bufs=2) as outp,
            tc.tile_pool(name="small", bufs=4) as small,
            tc.tile_pool(name="tpsum", bufs=2, space="PSUM") as tpsum,
            tc.tile_pool(name="gpsum", bufs=2, space="PSUM") as gpsum,
            tc.tile_pool(name="opsum", bufs=1, space="PSUM") as opsum,
        ):
            ident_f32 = consts.tile([P, P], f32)
            make_identity(nc, ident_f32[:])
            ident_bf16 = consts.tile([P, P], bf16)
            make_identity(nc, ident_bf16[:])

            S_T = big.tile([P, KD, T], f32r)      # [d%128, (dblk, m)]
            S_nat = big.tile([P, LT, DH], bf16)   # [m%128, (mblk, dcol)]
            D_nat = big.tile([P, LT, DH], bf16)   # [l%128, (lblk, dcol)]
            GT = dram.tile([T, T], f32)           # G^T in DRAM, [m, l]

            # ---- Phase A: S loads, S^T fp32r + S_nat bf16 ----
            for i in range(LT):
                st = stage.tile([P, DH], f32, tag="ld")
                nc.sync.dma_start(st[:], S_ap[i * P:(i + 1) * P, :])
                nc.vector.tensor_copy(S_nat[:, i, :], st[:])
                for g in range(2):
                    pt = tpsum.tile([P, 4, P], f32, tag="tp")
                    for k4 in range(4):
                        k = g * 4 + k4
                        nc.tensor.transpose(
                            pt[:, k4, :], st[:, k * P:(k + 1) * P], ident_f32[:]
                        )
                    nc.vector.tensor_copy(
                        S_T[:, g * 4:(g + 1) * 4, i * P:(i + 1) * P], pt[:]
                    )

            # ---- Phase B: G blocks, exp, G^T export, O_D ----
            for i in range(LT):
                std = stage.tile([P, DH], f32, tag="ld")
                nc.sync.dma_start(std[:], D_ap[i * P:(i + 1) * P, :])
                nc.vector.tensor_copy(D_nat[:, i, :], std[:])
                dt_i = dtp.tile([P, KD, P], f32r)
                for g in range(2):
                    pt = tpsum.tile([P, 4, P], f32, tag="tp")
                    for k4 in range(4):
                        k = g * 4 + k4
                        nc.tensor.transpose(
                            pt[:, k4, :], std[:, k * P:(k + 1) * P], ident_f32[:]
                        )
                    nc.vector.tensor_copy(dt_i[:, g * 4:(g + 1) * 4, :], pt[:])

                g_sb = gsb.tile([P, T], f32)
                for mc in range(T // NTILE):
                    gp = gpsum.tile([P, NTILE], f32, tag="g")
                    for k in range(KD):
                        nc.tensor.matmul(
                            gp[:],
                            dt_i[:, k, :],
                            S_T[:, k, mc * NTILE:(mc + 1) * NTILE],
                            start=(k == 0),
                            stop=(k == KD - 1),
                        )
                    nc.scalar.copy(g_sb[:, mc * NTILE:(mc + 1) * NTILE], gp[:])

                nr = small.tile([P, 1], f32, tag="nr")
                nc.vector.reduce_max(nr[:], g_sb[:], axis=AX, negate=True)
                e_i = epool.tile([P, T], bf16, tag="e")
                rs = small.tile([P, 1], f32, tag="rs")
                nc.scalar.activation(
                    e_i[:], g_sb[:], EXP, bias=nr[:], scale=1.0, accum_out=rs[:]
                )
                rrs = small.tile([P, 1], f32, tag="rrs")
                nc.vector.reciprocal(rrs[:], rs[:])

                # export G^T tiles to DRAM for phase C
                for g in range(4):
                    ptg = tpsum.tile([P, 4, P], f32, tag="tp")
                    for j4 in range(4):
                        j = g * 4 + j4
                        nc.tensor.transpose(
                            ptg[:, j4, :], g_sb[:, j * P:(j + 1) * P], ident_f32[:]
                        )
                    gt_sb = gtsb.tile([P, 4, P], f32)
                    nc.scalar.copy(gt_sb[:], ptg[:])
                    nc.sync.dma_start(
                        GT[g * 4 * P:(g + 1) * 4 * P, i * P:(i + 1) * P].rearrange(
                            "(a p) c -> p a c", p=P
                        ),
                        gt_sb[:],
                    )

                # O_D: accumulate over m blocks
                od = opsum.tile([P, DH], f32, tag="o")
                for jg in range(4):
                    pte = tpsum.tile([P, 4, P], bf16, tag="tpe")
                    for j4 in range(4):
                        j = jg * 4 + j4
                        nc.tensor.transpose(
                            pte[:, j4, :], e_i[:, j * P:(j + 1) * P], ident_bf16[:]
                        )
                    et = etp.tile([P, 4, P], bf16, tag="et")
                    nc.vector.tensor_copy(et[:], pte[:])
                    for j4 in range(4):
                        j = jg * 4 + j4
                        for n in range(DH // NTILE):
                            nc.tensor.matmul(
                                od[:, n * NTILE:(n + 1) * NTILE],
                                et[:, j4, :],
                                S_nat[:, j, n * NTILE:(n + 1) * NTILE],
                                start=(j == 0),
                                stop=(j == LT - 1),
                            )
                o = outp.tile([P, DH], f32, tag="o")
                nc.scalar.activation(o[:], od[:], COPY, scale=rrs[:])
                nc.vector.tensor_add(o[:], o[:], std[:])
                nc.sync.dma_start(coD_ap[i * P:(i + 1) * P, :], o[:])

            # ---- Phase C: G^T rows -> col softmax -> O_S ----
            for j in range(LT):
                gst = gtld.tile([P, T], f32)
                nc.sync.dma_start(gst[:], GT[j * P:(j + 1) * P, :])
                sst = stage.tile([P, DH], f32, tag="ld")
                nc.sync.dma_start(sst[:], S_ap[j * P:(j + 1) * P, :])

                ncm = small.tile([P, 1], f32, tag="nr")
                nc.vector.reduce_max(ncm[:], gst[:], axis=AX, negate=True)
                e2 = epool.tile([P, T], bf16, tag="e")
                cs = small.tile([P, 1], f32, tag="rs")
                nc.scalar.activation(
                    e2[:], gst[:], EXP, bias=ncm[:], scale=1.0, accum_out=cs[:]
                )
                rcs = small.tile([P, 1], f32, tag="rrs")
                nc.vector.reciprocal(rcs[:], cs[:])

                os_ = opsum.tile([P, DH], f32, tag="o")
                for ig in range(4):
                    pte = tpsum.tile([P, 4, P], bf16, tag="tpe")
                    for i4 in range(4):
                        i = ig * 4 + i4
                        nc.tensor.transpose(
                            pte[:, i4, :], e2[:, i * P:(i + 1) * P], ident_bf16[:]
                        )
                    et = etp.tile([P, 4, P], bf16, tag="et")
                    nc.vector.tensor_copy(et[:], pte[:])
                    for i4 in range(4):
                        i = ig * 4 + i4
                        for n in range(DH // NTILE):
                            nc.tensor.matmul(
                                os_[:, n * NTILE:(n + 1) * NTILE],
                                et[:, i4, :],
                                D_nat[:, i, n * NTILE:(n + 1) * NTILE],
                                start=(i == 0),
                                stop=(i == LT - 1),
                            )
                o = outp.tile([P, DH], f32, tag="o")
                nc.scalar.activation(o[:], os_[:], COPY, scale=rcs[:])
                nc.vector.tensor_add(o[:], o[:], sst[:])
                nc.sync.dma_start(coS_ap[j * P:(j + 1) * P, :], o[:])

    nc.compile()
    return nc


def _get_nc():
    if "nc" not in _CACHE:
        _CACHE["nc"] = _build_nc()
    return _CACHE["nc"]


def kernel(S, D):
    from concourse.bass_utils import run_bass_kernel_spmd

    S = np.ascontiguousarray(np.asarray(S, dtype=np.float32))
    D = np.ascontiguousarray(np.asarray(D, dtype=np.float32))
    B = S.shape[0]
    assert S.shape == (B, T, DH) and D.shape == (B, T, DH) and B == 8

    nc = _get_nc()
    in_maps = [{"S": S[b], "D": D[b]} for b in range(B)]
    res = run_bass_kernel_spmd(nc, in_maps, core_ids=list(range(B)))
    co_D = np.stack([res.results[b]["co_D"] for b in range(B)])
    co_S = np.stack([res.results[b]["co_S"] for b in range(B)])
    return (co_D, co_S)


# revision 16
# speedup vs baseline: 1.1244x; 1.1244x over previous
"""CoAttention kernel for 8 Trainium2 NeuronCores.

Problem: S, D: [8, 2048, 1024] f32.
  G_b = D_b @ S_b^T                         [2048, 2048]
  co_D = D + rowsoftmax(G) @ S
  co_S = S + rowsoftmax(G^T) @ D
Data-parallel over batch: one batch per core, same NEFF on all 8 cores.

Per-core algorithm (all loops fully unrolled, Tile-scheduled):
  Phase A: load S; keep S^T (fp32r, for stage-1 rhs) and S_nat (16-bit,
           stage-2 rhs) in SBUF.
  Phase B (per 128-row l-block): load D block, PE-transpose to fp32r
           D^T tiles; 8x accumulate fp32r matmuls -> G block; row max;
           exp (ACT, accum_out gives rowsum); PE-transpose the fp32 G
           block out to a DRAM G^T buffer for phase C; then
           O_D = (E/rowsum) @ S via PE-transposed 16-bit E tiles,
           + D residual -> co_D.
  Phase C (per 128-row m-block of G^T): load G^T rows, col max, exp,
           O_S = (E2/colsum) @ D via PE-transposed 16-bit E2 tiles,
           + S residual -> co_S.
fp32 logits throughout; 16-bit only on post-softmax attention weights
and stage-2 operands. Softmax uses exact per-row / per-column maxes.
"""

import numpy as np

P = 128
T = 2048
DH = 1024
LT = T // P     # 16 token blocks per side
KD = DH // P    # 8 contraction blocks
NTILE = 512     # matmul moving free dim

# tuning knobs (defaults = shipping config)
DEFAULTS = dict(
    e_dtype="fp16",       # dtype of E tiles / S_nat / D_nat (stage-2 operands)
    chunked_redmax=True,  # rowmax per G chunk instead of one big reduce
    chunked_exp=True,     # exp per 512-col chunk instead of one big activation
    use_gpsimd=True,      # casts + residual adds on GpSimd
    g_copy_eng="dve",     # engine for G psum->sbuf copies
    dma_transpose_e=False, # E-tile transposes on DMA xbar instead of PE+copy
    stage_bufs=3,
    gsb_bufs=2,
    epool_bufs=2,
    etp_bufs=3,
    gtsb_bufs=2,
    outp_bufs=2,
    gpsum_bufs=2,
    opsum_bufs=1,
    tpsum_bufs=2,
)

_CACHE = {}


def _build_nc(**overrides):
    import concourse.bass as bass
    import concourse.mybir as mybir
    import concourse.tile as tile
    from concourse import bacc
    from concourse.masks import make_identity

    p = dict(DEFAULTS)
    p.update(overrides)

    dt = mybir.dt
    f32, f32r = dt.float32, dt.float32r
    e16 = dt.float16 if p["e_dtype"] == "fp16" else dt.bfloat16
    AX = mybir.AxisListType.X
    EXP = mybir.ActivationFunctionType.Exp
    COPY = mybir.ActivationFunctionType.Copy
    MAX = mybir.AluOpType.max

    nc = bacc.Bacc("TRN2", target_bir_lowering=False, debug=False)

    S_ap = nc.dram_tensor("S", [T, DH], f32, kind="ExternalInput").ap()
    D_ap = nc.dram_tensor("D", [T, DH], f32, kind="ExternalInput").ap()
    coD_ap = nc.dram_tensor("co_D", [T, DH], f32, kind="ExternalOutput").ap()
    coS_ap = nc.dram_tensor("co_S", [T, DH], f32, kind="ExternalOutput").ap()

    NCH = T // NTILE  # 4 chunks per token row

    with tile.TileContext(nc) as tc:
        with (
            tc.tile_pool(name="consts", bufs=1) as consts,
            tc.tile_pool(name="big", bufs=1) as big,
            tc.tile_pool(name="dram", bufs=1, space="DRAM") as dram,
            tc.tile_pool(name="stage", bufs=p["stage_bufs"]) as stage,
            tc.tile_pool(name="epool", bufs=p["epool_bufs"]) as epool,
            tc.tile_pool(name="etp", bufs=max(6, p["etp_bufs"])) as etp,
            tc.tile_pool(name="gtsb", bufs=p["gtsb_bufs"]) as gtsb,
            tc.tile_pool(name="outp", bufs=p["outp_bufs"]) as outp,
            tc.tile_pool(name="small", bufs=4) as small,
            tc.tile_pool(name="tpsum", bufs=p["tpsum_bufs"], space="PSUM") as tpsum,
        ):
            ident_f32 = consts.tile([P, P], f32)
            make_identity(nc, ident_f32[:])
            ident_e16 = consts.tile([P, P], e16)
            make_identity(nc, ident_e16[:])

            S_T = big.tile([P, KD, T], f32r)      # [d%128, (dblk, m)]
            S_nat = big.tile([P, LT, DH], e16)    # [m%128, (mblk, dcol)]
            D_nat = big.tile([P, LT, DH], e16)    # [l%128, (lblk, dcol)]
            GT = dram.tile([T, T], f32)           # G^T in DRAM, [m, l]

            def softmax_row(g_sb, rmax_parts):
                """-rowmax -> exp -> E (e16) + rowsum + recip.

                Returns (e, rrs, ets): ets is the list of 4 transposed
                E-tile groups when the DMA-xbar path is on (each produced
                by an ACT-issued transpose DMA right after its exp chunk),
                else None."""
                nr = small.tile([P, 1], f32, tag="nr")
                if rmax_parts is not None:
                    nc.vector.reduce_max(nr[:], rmax_parts[:], axis=AX, negate=True)
                else:
                    nc.vector.reduce_max(nr[:], g_sb[:], axis=AX, negate=True)
                e = epool.tile([P, T], e16, tag="e")
                rs = small.tile([P, 1], f32, tag="rs")
                ets = [] if p["dma_transpose_e"] else None
                if p["chunked_exp"]:
                    rsp = small.tile([P, NCH], f32, tag="rsp", name="rsp")
                    for mc in range(NCH):
                        sl = slice(mc * NTILE, (mc + 1) * NTILE)
                        nc.scalar.activation(
                            e[:, sl], g_sb[:, sl], EXP, bias=nr[:], scale=1.0,
                            accum_out=rsp[:, mc:mc + 1],
                        )
                        if ets is not None:
                            et = etp.tile([P, 4, P], e16, tag="et", name="et")
                            nc.scalar.dma_start_transpose(et[:], e[:, sl])
                            ets.append(et)
                    nc.vector.reduce_sum(rs[:], rsp[:], axis=AX)
                else:
                    nc.scalar.activation(
                        e[:], g_sb[:], EXP, bias=nr[:], scale=1.0, accum_out=rs[:]
                    )
                    if ets is not None:
                        for mc in range(NCH):
                            et = etp.tile([P, 4, P], e16, tag="et", name="et")
                            nc.scalar.dma_start_transpose(
                                et[:], e[:, mc * NTILE:(mc + 1) * NTILE]
                            )
                            ets.append(et)
                rrs = small.tile([P, 1], f32, tag="rrs")
                nc.vector.reciprocal(rrs[:], rs[:])
                return e, rrs, ets

            def stage2(e, rhs_big, ps_tag, ets=None, pool=None):
                """O += E^T-tiles @ rhs over 16 K blocks. Returns psum [P, DH]."""
                ps = (pool or opsum).tile([P, DH], f32, tag=ps_tag, name="ps")
                for kg in range(4):
                    if ets is not None:
                        et = ets[kg]
                    else:
                        pte = tpsum.tile([P, 4, P], e16, tag="tpe")
                        for k4 in range(4):
                            kb = kg * 4 + k4
                            nc.tensor.transpose(
                                pte[:, k4, :], e[:, kb * P:(kb + 1) * P], ident_e16[:]
                            )
                        et = etp.tile([P, 4, P], e16, tag="et")
                        nc.vector.tensor_copy(et[:], pte[:])
                    for k4 in range(4):
                        kb = kg * 4 + k4
                        for n in range(DH // NTILE):
                            nc.tensor.matmul(
                                ps[:, n * NTILE:(n + 1) * NTILE],
                                et[:, k4, :],
                                rhs_big[:, kb, n * NTILE:(n + 1) * NTILE],
                                start=(kb == 0),
                                stop=(kb == LT - 1),
                            )
                return ps

            def emit_out(ps, rscale, resid, out_ap):
                o = outp.tile([P, DH], f32, tag="o")
                nc.scalar.activation(o[:], ps[:], COPY, scale=rscale[:])
                adder = nc.gpsimd if p["use_gpsimd"] else nc.vector
                adder.tensor_add(o[:], o[:], resid[:])
                dma_eng = nc.gpsimd if p["use_gpsimd"] else nc.sync
                dma_eng.dma_start(out_ap, o[:])

            # ---- Phase A: S loads, S^T fp32r + S_nat ----
            gpsum_ctx = tc.tile_pool(name="gpsum", bufs=p["gpsum_bufs"], space="PSUM")
            gpsum = gpsum_ctx.__enter__()
            opsum_ctx = tc.tile_pool(name="opsum", bufs=p["opsum_bufs"], space="PSUM")
            opsum = opsum_ctx.__enter__()
            ab_ctx = tc.tile_pool(name="dtp", bufs=2)
            dtp = ab_ctx.__enter__()
            gsb_ctx = tc.tile_pool(name="gsb", bufs=p["gsb_bufs"])
            gsb = gsb_ctx.__enter__()
            gtsb_ctx = tc.tile_pool(name="gtsb", bufs=p["gtsb_bufs"])
            gtsb = gtsb_ctx.__enter__()
            st_tiles = {}
            for i in range(2):
                st_tiles[i] = stage.tile([P, DH], f32, tag="ld", name="st")
                nc.sync.dma_start(st_tiles[i][:], S_ap[i * P:(i + 1) * P, :])
            for i in range(LT):
                if i + 2 < LT:
                    st_tiles[i + 2] = stage.tile([P, DH], f32, tag="ld", name="st")
                    nc.sync.dma_start(
                        st_tiles[i + 2][:], S_ap[(i + 2) * P:(i + 3) * P, :]
                    )
                st = st_tiles.pop(i)
                caster = nc.gpsimd if p["use_gpsimd"] else nc.vector
                caster.tensor_copy(S_nat[:, i, :], st[:])
                for g in range(2):
                    pt = tpsum.tile([P, 4, P], f32, tag="tp")
                    for k4 in range(4):
                        k = g * 4 + k4
                        nc.tensor.transpose(
                            pt[:, k4, :], st[:, k * P:(k + 1) * P], ident_f32[:]
                        )
                    nc.vector.tensor_copy(
                        S_T[:, g * 4:(g + 1) * 4, i * P:(i + 1) * P], pt[:]
                    )

            # ---- Phase B: G blocks, exp, G^T export, O_D ----
            std_tiles = {}
            for i in range(2):
                std_tiles[i] = stage.tile([P, DH], f32, tag="ld", name="std")
                nc.sync.dma_start(std_tiles[i][:], D_ap[i * P:(i + 1) * P, :])
            for i in range(LT):
                if i + 2 < LT:
                    std_tiles[i + 2] = stage.tile([P, DH], f32, tag="ld", name="std")
                    nc.sync.dma_start(
                        std_tiles[i + 2][:], D_ap[(i + 2) * P:(i + 3) * P, :]
                    )
                std = std_tiles.pop(i)
                caster = nc.gpsimd if p["use_gpsimd"] else nc.vector
                caster.tensor_copy(D_nat[:, i, :], std[:])
                dt_i = dtp.tile([P, KD, P], f32r)
                for g in range(2):
                    pt = tpsum.tile([P, 4, P], f32, tag="tp")
                    for k4 in range(4):
                        k = g * 4 + k4
                        nc.tensor.transpose(
                            pt[:, k4, :], std[:, k * P:(k + 1) * P], ident_f32[:]
                        )
                    nc.vector.tensor_copy(dt_i[:, g * 4:(g + 1) * 4, :], pt[:])

                g_sb = gsb.tile([P, T], f32)
                if p["chunked_redmax"]:
                    rmp = small.tile([P, NCH], f32, tag="rmp", name="rmp")
                else:
                    rmp = None
                for mc in range(NCH):
                    gp = gpsum.tile([P, NTILE], f32, tag="g")
                    for k in range(KD):
                        nc.tensor.matmul(
                            gp[:],
                            dt_i[:, k, :],
                            S_T[:, k, mc * NTILE:(mc + 1) * NTILE],
                            start=(k == 0),
                            stop=(k == KD - 1),
                        )
                    if p["g_copy_eng"] == "dve":
                        nc.vector.tensor_copy(g_sb[:, mc * NTILE:(mc + 1) * NTILE], gp[:])
                    else:
                        nc.scalar.copy(g_sb[:, mc * NTILE:(mc + 1) * NTILE], gp[:])
                    if rmp is not None:
                        nc.vector.tensor_reduce(
                            rmp[:, mc:mc + 1],
                            g_sb[:, mc * NTILE:(mc + 1) * NTILE],
                            axis=AX, op=MAX,
                        )
                e_i, rrs, ets = softmax_row(g_sb, rmp)

                # export G^T tiles to DRAM for phase C
                for g in range(4):
                    ptg = tpsum.tile([P, 4, P], f32, tag="tp")
                    for j4 in range(4):
                        j = g * 4 + j4
                        nc.tensor.transpose(
                            ptg[:, j4, :], g_sb[:, j * P:(j + 1) * P], ident_f32[:]
                        )
                    gt_sb = gtsb.tile([P, 4, P], f32)
                    nc.scalar.copy(gt_sb[:], ptg[:])
                    nc.scalar.dma_start(
                        GT[g * 4 * P:(g + 1) * 4 * P, i * P:(i + 1) * P].rearrange(
                            "(a p) c -> p a c", p=P
                        ),
                        gt_sb[:],
                    )

                od = stage2(e_i, S_nat, "o", ets)
                emit_out(od, rrs, std, coD_ap[i * P:(i + 1) * P, :])

            gtsb_ctx.__exit__(None, None, None)
            gsb_ctx.__exit__(None, None, None)
            ab_ctx.__exit__(None, None, None)
            opsum_ctx.__exit__(None, None, None)
            gpsum_ctx.__exit__(None, None, None)
            opsum_c_ctx = tc.tile_pool(name="opsum_c", bufs=2, space="PSUM")
            opsum_c = opsum_c_ctx.__enter__()
            gtld_ctx = tc.tile_pool(name="gtld", bufs=3)
            gtld = gtld_ctx.__enter__()

            # ---- Phase C: G^T rows -> col softmax -> O_S ----
            def load_c(j):
                g = gtld.tile([P, T], f32, name="gst")
                for mc in range(NCH):
                    nc.sync.dma_start(
                        g[:, mc * NTILE:(mc + 1) * NTILE],
                        GT[j * P:(j + 1) * P, mc * NTILE:(mc + 1) * NTILE],
                    )
                s = stage.tile([P, DH], f32, tag="ld", name="sst")
                nc.sync.dma_start(s[:], S_ap[j * P:(j + 1) * P, :])
                return g, s

            c_tiles = {}
            for j in range(2):
                c_tiles[j] = load_c(j)
            for j in range(LT):
                if j + 2 < LT:
                    c_tiles[j + 2] = load_c(j + 2)
                gst, sst = c_tiles.pop(j)
                cmp_ = small.tile([P, NCH], f32, tag="rmp", name="cmp_")
                for mc in range(NCH):
                    nc.vector.tensor_reduce(
                        cmp_[:, mc:mc + 1],
                        gst[:, mc * NTILE:(mc + 1) * NTILE],
                        axis=AX, op=MAX,
                    )
                e2, rcs, ets2 = softmax_row(gst, cmp_)
                os_ = stage2(e2, D_nat, "oc", ets2, pool=opsum_c)
                emit_out(os_, rcs, sst, coS_ap[j * P:(j + 1) * P, :])
            gtld_ctx.__exit__(None, None, None)
            opsum_c_ctx.__exit__(None, None, None)

    nc.compile()
    return nc


def _get_nc():
    if "nc" not in _CACHE:
        _CACHE["nc"] = _build_nc()
    return _CACHE["nc"]


def kernel(S, D):
    from concourse.bass_utils import run_bass_kernel_spmd

    S = np.ascontiguousarray(np.asarray(S, dtype=np.float32))
    D = np.ascontiguousarray(np.asarray(D, dtype=np.float32))
    B = S.shape[0]
    assert S.shape == (B, T, DH) and D.shape == (B, T, DH) and B == 8

    nc = _get_nc()
    in_maps = [{"S": S[b], "D": D[b]} for b in range(B)]
    res = run_bass_kernel_spmd(nc, in_maps, core_ids=list(range(B)))
    co_D = np.stack([res.results[b]["co_D"] for b in range(B)])
    co_S = np.stack([res.results[b]["co_S"] for b in range(B)])
    return (co_D, co_S)
